# revision 48
# baseline (speedup 1.0000x reference)
"""Trainium2 Bass kernel for GQA attention (B=2,T=2048,D=2048,Hq=16,Hkv=4,Dh=128)
with RMSNorm + YaRN RoPE on q/k, causal softmax, out projection.

Sharding: DP(batch) x TP(heads) hybrid across 8 cores.
  core c -> batch g = c//4, kv group r = c%4, query heads {r, r+4, r+8, r+12}.
  Each core projects q (4 heads) + k + v (its group) for its batch only,
  runs RMSNorm+RoPE, causal attention for its 4 heads, chunk-pipelined with
  the projection.  Two 8-core AllToAlls (one per 8-block half of T)
  redistribute attention outputs from head-sharded to token-sharded: core c
  ends up owning token blocks {c, c+8} of BOTH batches and runs the
  out-projection against full Wo.  Host concatenates token slices.
  Compute in bf16 (f32 PSUM).

v2 scheduling (from trace analysis of v1):
  - gs loads ride gpsimd only: a DMA waiting on the collective must never
    sit on the Scalar queue where it blocks the attention exps.
  - Wo streams in as 8 x 1MB pieces on the (otherwise idle after startup)
    Scalar DMA queue, spread across chunks 0-1, so it neither bursts the
    HBM nor delays the half-0 a2a-input DMAs.
  - proj/rope software pipeline: DVE rope runs right after the projection
    matmuls of its block, but the PE transposes of the roped outputs are
    deferred one chunk so they never wait on the DVE chain.
  - rope processes all 4 query heads in single wide DVE ops (broadcast
    APs for the cos/sin tables and the per-head rsqrt); k-rope, the RMS
    square-accumulates, and all small copies run on the idle Pool
    (gpsimd) engine.
  - score rows are computed in pairs into one 2-bank PSUM tile and
    exponentiated with a single [128,1024] ACT instruction (the causal
    mask is added in-place in PSUM first), halving ACT instruction count.
"""

import math

import numpy as np
import ml_dtypes

import concourse.bass as bass
import concourse.tile as tile
from concourse import bacc, mybir
from concourse.bass_utils import run_bass_kernel_spmd
from concourse.masks import make_identity

# ---- problem constants --------------------------------------------------
B = 2
T = 2048
D_MODEL = 2048
D_HEAD = 128
N_Q, N_KV = 16, 4
ROPE_BASE = 10000.0
YARN_SCALE = 4.0
ORIG_MAX_LEN = 4096
BETA_FAST, BETA_SLOW = 32.0, 1.0
RMS_EPS = 1.1920929e-07
MSCALE = 0.1 * math.log(YARN_SCALE) + 1.0
ATTN_SCALE = 1.0 / (MSCALE * math.sqrt(D_HEAD))

N_CORES = 8
GROUP = 4                    # cores per batch (TP group size)
NH = N_Q // GROUP            # query heads per core
KT = D_MODEL // 128          # 16 contraction tiles
QC = 4                       # query blocks per attention chunk
BF16 = mybir.dt.bfloat16
F32 = mybir.dt.float32
NEG_BIG = -1e30
Alu = mybir.AluOpType
Act = mybir.ActivationFunctionType

# oproj: gs slot idx = s*NH + hh holds global head 4*hh + s
HEAD_OF = [4 * (idx % NH) + idx // NH for idx in range(GROUP * NH)]


# ---- bass graph ---------------------------------------------------------

def build_nc(tb_count=T // 128):
    """One SPMD graph shared by all 8 cores."""
    TB = tb_count
    assert TB % 8 == 0 and TB % QC == 0
    NJ = TB // QC                  # attention chunks
    NHALF = TB // 8                # a2a halves
    t_tokens = TB * 128
    nc = bacc.Bacc("TRN2", target_bir_lowering=False, debug=False,
                   num_devices=N_CORES)

    # all inputs are pre-tiled on the host into exactly the SBUF layouts so
    # every DMA reads large contiguous spans (256B-element gathers run at
    # ~1/3 of HBM bandwidth and starve the PE's SBUF ports)
    xT = nc.dram_tensor("xT", [TB, 128, KT, 128], BF16, kind="ExternalInput")
    wc = nc.dram_tensor("wc", [128, KT, 768], BF16, kind="ExternalInput")
    wo = nc.dram_tensor("wo", [4, 2, 128, 8, 512], BF16,
                        kind="ExternalInput")
    cosq = nc.dram_tensor("cosq", [128, TB, 128], BF16, kind="ExternalInput")
    sinq = nc.dram_tensor("sinq", [128, TB, 128], BF16, kind="ExternalInput")
    cosk = nc.dram_tensor("cosk", [128, TB, 128], BF16, kind="ExternalInput")
    sink = nc.dram_tensor("sink", [128, TB, 128], BF16, kind="ExternalInput")
    maskt = nc.dram_tensor("maskt", [128, 128], F32, kind="ExternalInput")
    out = nc.dram_tensor("out", [NHALF, B, 128, D_MODEL], BF16,
                         kind="ExternalOutput")

    with tile.TileContext(nc) as tc:
        with (
            tc.tile_pool(name="const", bufs=1) as constp,
            tc.tile_pool(name="xin", bufs=2) as xin,
            tc.tile_pool(name="qnp", bufs=8) as qnp,
            tc.tile_pool(name="knp", bufs=8) as knp,
            tc.tile_pool(name="persist", bufs=1) as persist,
            tc.tile_pool(name="work", bufs=2) as work,
            tc.tile_pool(name="outp", bufs=1) as outp,
            tc.tile_pool(name="psum", bufs=1, space="PSUM") as psum,
            tc.tile_pool(name="dram", bufs=1, space="DRAM") as dram,
        ):
            # ---- constants ------------------------------------------------
            # wc first, 4 back-to-back DMAs on the scalar queue (its HW DMA
            # queue is otherwise unused); xt for chunk 0 on sync in parallel
            # tables: first chunk's rows early (small), tails deferred
            cs = {}
            tbl_tails = []
            for name, t in (("cosq", cosq), ("sinq", sinq),
                            ("cosk", cosk), ("sink", sink)):
                s = constp.tile([128, TB, 128], BF16, tag=name, name=name)
                tv = t.ap()
                nc.gpsimd.dma_start(out=s[:, 0:QC, :], in_=tv[:, 0:QC, :])
                tbl_tails.append((s, tv))
                cs[name] = s
            wcs = constp.tile([128, KT, 768], BF16, tag="wc", name="wc")
            wc_dmas = []
            for g in range(KT):
                eng = nc.scalar if g % 2 == 0 else nc.gpsimd
                wd = eng.dma_start(
                    out=wcs[:, g:g + 1, :],
                    in_=wc.ap()[:, g:g + 1, :])
                wc_dmas.append(wd)
            for i, (s, tv) in enumerate(tbl_tails):
                td = nc.gpsimd.dma_start(out=s[:, QC:TB, :],
                                         in_=tv[:, QC:TB, :])
                if i == 0:
                    tile.add_dep_helper(td.ins, wc_dmas[-2].ins, sync=True,
                                        reason="defer table tails")
            mask_sb = constp.tile([128, 128], F32, tag="mask", name="mask")
            nc.gpsimd.dma_start(out=mask_sb, in_=maskt.ap())
            ident = constp.tile([128, 128], BF16, tag="ident", name="ident")
            make_identity(nc, ident)
            eps_sb = constp.tile([128, 1], F32, tag="eps", name="eps")
            nc.vector.memset(eps_sb, RMS_EPS)

            # persistent attention state
            qTall = persist.tile([128, NH, TB * 128], BF16, tag="qT",
                                 name="qT")
            kTt = persist.tile([128, TB, 128], BF16, tag="kT", name="kT")
            vA = persist.tile([128, TB, 129], BF16, tag="vA", name="vA")
            nc.vector.memset(vA[:, :, 128:129], 1.0)

            # full Wo resident in SBUF (4 col-chunks); streamed in as 8
            # 1MB pieces on the scalar queue at points spread over chunks 0-1
            wo_t = [constp.tile([128, N_Q, 512], BF16, tag=f"wo{cc}",
                                name=f"wo{cc}") for cc in range(4)]
            wo_pieces = [(cc, hlf) for cc in range(4) for hlf in range(2)]

            def fire_wo_piece():
                if not wo_pieces:
                    return
                cc, hlf = wo_pieces.pop(0)
                nc.scalar.dma_start(
                    out=wo_t[cc][:, 8 * hlf:8 * (hlf + 1), :],
                    in_=wo.ap()[cc, hlf])

            # ---- A2A bounce buffers (per half) ---------------------------
            a2a_in, a2a_out = {}, {}
            for h in range(NHALF):
                a2a_in[h] = dram.tile([N_CORES, NH, 128, 128], BF16,
                                      tag=f"a2ain{h}", name=f"a2ain{h}")
                a2a_out[h] = dram.tile([N_CORES, NH, 128, 128], BF16,
                                       tag=f"a2aout{h}", name=f"a2aout{h}")

            # ---- projection matmuls + rope (DVE part) for one block ------
            xt_tiles = {}

            def prefetch_xt_chunk(j):
                # half-chunk (2-block) granularity: big enough DMAs, small
                # enough tiles; chunk 0 loads per-block so the first matmul
                # starts asap
                for half in range(2):
                    key = 2 * j + half
                    t0 = j * QC + 2 * half
                    if key in xt_tiles or t0 >= TB:
                        continue
                    xt = xin.tile([128, 2, KT, 128], BF16, tag="xt",
                                  name="xt", bufs=3)
                    if j == 0:
                        for q in range(2):
                            nc.sync.dma_start(out=xt[:, q],
                                              in_=xT.ap()[t0 + q])
                    else:
                        nc.sync.dma_start(
                            out=xt,
                            in_=xT.ap()[t0:t0 + 2]
                                .rearrange("tb p f c -> p tb f c"))
                    xt_tiles[key] = xt

            qn_tiles, kn_tiles = {}, {}

            def proj_mm(tb):
                xt = xt_tiles[tb // 2]
                xtb = tb % 2
                psq = psum.tile([128, NH, 128], F32, tag="pq", name="pq",
                                bufs=2)
                pskv = psum.tile([128, 2, 128], F32, tag="rb", name="pkv",
                                 bufs=2)
                for kt in range(KT):
                    nc.tensor.matmul(psq[:, :, :], xt[:, xtb, kt, :],
                                     wcs[:, kt, 0:512],
                                     start=(kt == 0), stop=(kt == KT - 1))
                for kt in range(KT):
                    nc.tensor.matmul(pskv[:, :, :], xt[:, xtb, kt, :],
                                     wcs[:, kt, 512:768],
                                     start=(kt == 0), stop=(kt == KT - 1))
                # RMS rsqrt: squares+row-sums on ACT (gpsimd can't read
                # PSUM), sqrt ACT, recip DVE; k/v leave PSUM via ACT copies
                sqscr = work.tile([128, 128], BF16, tag="sqscr",
                                   name="sqscr", bufs=1)
                ssq = work.tile([128, 5], F32, tag="ssq", name="ssq")
                for i in range(NH):
                    nc.scalar.activation(sqscr, psq[:, i, :], Act.Square,
                                         accum_out=ssq[:, i:i + 1])
                nc.scalar.activation(sqscr, pskv[:, 0, :], Act.Square,
                                     accum_out=ssq[:, 4:5])
                nc.scalar.activation(vA[:, tb, 0:128], pskv[:, 1, :],
                                     Act.Copy)
                std = work.tile([128, 5], F32, tag="std", name="std")
                nc.scalar.activation(std, ssq, Act.Sqrt,
                                     bias=eps_sb, scale=1.0 / 128.0)
                rv = work.tile([128, 5], F32, tag="rv", name="rv")
                nc.vector.reciprocal(rv, std)
                # normalized k to SBUF in one ACT copy (scale = rsqrt)
                ksb = work.tile([128, 128], BF16, tag="ksb", name="ksb")
                nc.scalar.activation(ksb, pskv[:, 0, :], Act.Copy,
                                     scale=rv[:, 4:5])
                # q rope: all 4 heads in wide DVE ops
                rvb = rv[:, 0:4, None].broadcast_to((128, 4, 128))
                cosb = cs["cosq"][:, None, tb, :].broadcast_to((128, 4, 128))
                sint = cs["sinq"][:, tb, :]
                qrv = work.tile([128, 4, 128], BF16, tag="qrv",
                                 name="qrv", bufs=1)
                nc.vector.tensor_tensor(qrv, psq[:, :, :], rvb, Alu.mult)
                t1 = work.tile([128, 4, 128], BF16, tag="t1", name="t1",
                                bufs=1)
                nc.vector.tensor_tensor(t1, qrv, cosb, Alu.mult)
                t2 = work.tile([128, 4, 128], BF16, tag="t2", name="t2",
                                bufs=1)
                nc.vector.tensor_tensor(
                    t2[:, :, 0:64], qrv[:, :, 64:128],
                    sint[:, None, 0:64].broadcast_to((128, 4, 64)), Alu.mult)
                nc.vector.tensor_tensor(
                    t2[:, :, 64:128], qrv[:, :, 0:64],
                    sint[:, None, 64:128].broadcast_to((128, 4, 64)),
                    Alu.mult)
                qn = qnp.tile([128, 4, 128], BF16, tag="qn", name="qn")
                nc.vector.tensor_tensor(qn, t1, t2, Alu.add)
                qn_tiles[tb] = qn
                # k rope on gpsimd from the normalized SBUF copy
                t1k = work.tile([128, 128], BF16, tag="t1k", name="t1k")
                nc.gpsimd.tensor_tensor(t1k, ksb, cs["cosk"][:, tb, :],
                                        Alu.mult)
                t2k = work.tile([128, 128], BF16, tag="t2k", name="t2k")
                nc.gpsimd.tensor_tensor(t2k[:, 0:64], ksb[:, 64:128],
                                        cs["sink"][:, tb, 0:64], Alu.mult)
                nc.gpsimd.tensor_tensor(t2k[:, 64:128], ksb[:, 0:64],
                                        cs["sink"][:, tb, 64:128], Alu.mult)
                kn = knp.tile([128, 128], BF16, tag="kn", name="kn")
                nc.gpsimd.tensor_tensor(kn, t1k, t2k, Alu.add)
                kn_tiles[tb] = kn

            # ---- deferred PE transposes for one block --------------------
            def rope_transpose(tb):
                qn = qn_tiles.pop(tb)
                trp = psum.tile([128, 4, 128], BF16, tag="rb", name="trp",
                                bufs=2)
                for hh in range(NH):
                    nc.tensor.transpose(trp[:, hh, :], qn[:, hh, :], ident)
                nc.vector.tensor_copy(
                    qTall[:, :, tb * 128:(tb + 1) * 128], trp[:, :, :])
                kn = kn_tiles.pop(tb)
                trpk = psum.tile([128, 128], BF16, tag="rb", name="trpk",
                                 bufs=2)
                nc.tensor.transpose(trpk, kn, ident)
                nc.vector.tensor_copy(kTt[:, tb, :], trpk)

            # ---- attention chunk j for head hh ---------------------------
            # Score rows are processed in PAIRS: both rows' matmuls land in
            # one [128,2,512] PSUM tile (2 banks), the causal mask is added
            # in place, then a single [128,1024] exp produces the pair's
            # probabilities.  Two-pass pa accumulators as in v1: pass A
            # runs pv for q-blocks 0,1 while scoring, pass B replays pv for
            # q-blocks 2,3 from the saved exp tiles.
            def attn_block(j, hh):
                q0 = j * QC
                nrows = q0 + QC
                qcols = qTall[:, hh, q0 * 128:(q0 + QC) * 128]
                pas = {}

                def alloc_pa(qq):
                    pas[qq] = psum.tile([128, 129], F32, tag="rb", name="pa",
                                        bufs=2)

                def score_pair(p):
                    ss = psum.tile([128, 2, 512], F32, tag="ps", name="ps",
                                   bufs=2)
                    for par in range(2):
                        kb = 2 * p + par
                        diag = kb >= q0
                        w = (q0 + QC - kb) if diag else QC
                        cols = qTall[:, hh, (kb if diag else q0) * 128:
                                     (q0 + QC) * 128]
                        nc.tensor.matmul(ss[:, par, 0:w * 128], kTt[:, kb, :],
                                         cols, start=True, stop=True)
                        if diag:
                            nc.vector.scalar_tensor_tensor(
                                out=ss[:, par, 0:128], in0=ss[:, par, 0:128],
                                scalar=1.0, in1=mask_sb,
                                op0=Alu.bypass, op1=Alu.add)
                    ptw = work.tile([128, 2, 512], BF16, tag="ptw",
                                    name="ptw", bufs=8)
                    nc.scalar.activation(ptw[:, :, :], ss[:, :, :], Act.Exp,
                                         scale=ATTN_SCALE)
                    return ptw

                def pv(p, ptw, qlo, qhi):
                    for par in range(2):
                        kb = 2 * p + par
                        if kb >= nrows:
                            continue
                        lo = kb - q0 if kb >= q0 else 0
                        for qq in range(max(qlo, lo), qhi):
                            off = qq - lo
                            nc.tensor.matmul(
                                pas[qq],
                                ptw[:, par, off * 128:(off + 1) * 128],
                                vA[:, kb, :], start=(kb == 0),
                                stop=(q0 + qq == kb))

                at4 = work.tile([128, QC, 128], BF16, tag="attnT",
                                name="attnT", bufs=2)

                def tail(qq):
                    pa = pas[qq]
                    rs = work.tile([128, 1], F32, tag="rsum", name="rsum")
                    nc.vector.reciprocal(rs, pa[:, 128:129])
                    an = work.tile([128, 128], BF16, tag="attn_n",
                                   name="attn_n")
                    nc.vector.tensor_scalar_mul(an, pa[:, 0:128], rs)
                    tr = psum.tile([128, 128], BF16, tag="ps", name="trt",
                                   bufs=2)
                    nc.tensor.transpose(tr, an, ident)
                    nc.vector.tensor_copy(at4[:, qq, :], tr)

                def flush_tails():
                    # one DMA moves all 4 query blocks of this (chunk, head)
                    nc.sync.dma_start(
                        out=a2a_in[q0 // 8][4 * ((q0 // 4) % 2):
                                            4 * ((q0 // 4) % 2) + 4, hh]
                            .rearrange("s p t -> p s t"),
                        in_=at4)

                npairs = (nrows + 1) // 2
                alloc_pa(0)
                alloc_pa(1)
                ptws = {}
                for p in range(npairs):
                    ptws[p] = score_pair(p)
                    if p > 0:
                        pv(p - 1, ptws[p - 1], 0, 2)
                pv(npairs - 1, ptws[npairs - 1], 0, 2)
                tail(0)
                tail(1)
                alloc_pa(2)
                alloc_pa(3)
                for p in range(npairs):
                    pv(p, ptws[p], 2, 4)
                tail(2)
                tail(3)
                flush_tails()

            def fire_a2a(h):
                nc.gpsimd.collective_compute(
                    "AllToAll", Alu.bypass,
                    replica_groups=[list(range(N_CORES))],
                    ins=[a2a_in[h].opt()], outs=[a2a_out[h].opt()])

            # ---- out-projection for one half -----------------------------
            gs_tiles = {}

            def load_gs(h):
                for b in range(B):
                    g = persist.tile([128, GROUP, NH, 128], BF16,
                                     tag="gs", name="gs", bufs=3)
                    gs_tiles[(h, b)] = g
                    nc.gpsimd.dma_start(
                        out=g,
                        in_=a2a_out[h][GROUP * b:GROUP * (b + 1)]
                            .rearrange("s hh p t -> p (s hh) t"))

            def oproj(h):
                gs = {b: gs_tiles[(h, b)] for b in range(B)}
                ob = {b: outp.tile([128, 4, 512], BF16, tag=f"os{b}",
                                   name=f"os{b}") for b in range(B)}
                for cc in range(4):
                    for b in range(B):
                        po = psum.tile([128, 512], F32, tag="ps",
                                       name="po", bufs=2)
                        for idx in range(GROUP * NH):
                            nc.tensor.matmul(po,
                                             gs[b][:, idx // NH, idx % NH, :],
                                             wo_t[cc][:, HEAD_OF[idx], :],
                                             start=(idx == 0),
                                             stop=(idx == GROUP * NH - 1))
                        nc.scalar.activation(ob[b][:, cc, :], po, Act.Copy)
                for b in range(B):
                    nc.sync.dma_start(out=out.ap()[h, b], in_=ob[b])

            # ---- emission: iter it does [transposes(it-1)] [proj(it)]
            # [attn(it-1)]; a2a for a half fires when its last chunk ends.
            prefetch_xt_chunk(0)
            for it in range(NJ + 1):
                if it > 0:
                    for tb in range((it - 1) * QC, it * QC):
                        rope_transpose(tb)
                if it < NJ:
                    for tb in range(it * QC, (it + 1) * QC):
                        proj_mm(tb)
                    prefetch_xt_chunk(it + 1)
                if it > 0:
                    j = it - 1
                    for hh in range(NH):
                        attn_block(j, hh)
                        if j < 2:
                            fire_wo_piece()
                    if ((j + 1) * QC) % 8 == 0:
                        h = ((j + 1) * QC) // 8 - 1
                        fire_a2a(h)
                        load_gs(h)
                        if h == NHALF - 1:
                            for hq in range(NHALF):
                                oproj(hq)
    nc.compile()
    return nc


# ---- host side ----------------------------------------------------------

def _yarn_tables(t_tokens):
    inv = 1.0 / ROPE_BASE ** (np.arange(0, D_HEAD, 2, dtype=np.float32) / D_HEAD)
    wavelengths = 2.0 * math.pi / inv
    low_wl = ORIG_MAX_LEN / BETA_SLOW
    high_wl = ORIG_MAX_LEN / BETA_FAST
    gamma = np.clip((low_wl - wavelengths) / (low_wl - high_wl), 0.0, 1.0)
    inv_freq = (gamma * inv + (1.0 - gamma) * inv / YARN_SCALE).astype(np.float32)
    t = np.arange(t_tokens, dtype=np.float32)
    freqs = np.outer(t, inv_freq)                      # (T, 64)
    emb = np.concatenate([freqs, freqs], axis=-1)      # (T, 128)
    return np.cos(emb).astype(np.float32), np.sin(emb).astype(np.float32)


def _host_prep(x, Wq, Wkv, Wo, q_norm_w, k_norm_w, tb_count=T // 128):
    TB = tb_count
    t_tokens = TB * 128
    bf = ml_dtypes.bfloat16

    def tile_tokens(a):
        # (t_tokens, 128) f32 -> (128 p, TB, 128) pre-tiled
        return np.ascontiguousarray(
            a.reshape(TB, 128, 128).transpose(1, 0, 2)).astype(bf)

    xTb = []
    for b in range(B):
        xT = x[b, :t_tokens, :].T                          # (D, T)
        xt = xT.reshape(KT, 128, TB, 128).transpose(2, 1, 0, 3)
        xTb.append(np.ascontiguousarray(xt).astype(bf))    # (TB,128,KT,128)
    cos, sin = _yarn_tables(t_tokens)
    sinF = sin.copy()
    sinF[:, :64] *= -1.0
    # rms weight applies to x before rope; the sin term reads the *rotated*
    # input, so its weight index is the input position (rolled by 64).
    wq_roll = np.concatenate([q_norm_w[64:], q_norm_w[:64]])
    wk_roll = np.concatenate([k_norm_w[64:], k_norm_w[:64]])
    cosq = tile_tokens(cos * q_norm_w[None, :])
    sinq = tile_tokens(sinF * wq_roll[None, :])
    cosk = tile_tokens(cos * k_norm_w[None, :])
    sink = tile_tokens(sinF * wk_roll[None, :])
    maskt = np.where(np.arange(128)[:, None] <= np.arange(128)[None, :],
                     0.0, NEG_BIG).astype(np.float32)       # [k, q]
    Wk, Wv = Wkv[:, :N_KV * D_HEAD], Wkv[:, N_KV * D_HEAD:]
    # wo pieces: [cc, half, p, head, n] contiguous 1MB pieces
    wo_t = np.ascontiguousarray(
        Wo.astype(bf).reshape(2, 8, 128, 4, 512).transpose(3, 0, 2, 1, 4))
    in_maps = []
    for c in range(N_CORES):
        g, r = c // GROUP, c % GROUP
        wcols = np.concatenate(
            [Wq[:, h * 128:(h + 1) * 128]
             for h in (r, r + 4, r + 8, r + 12)]
            + [Wk[:, r * 128:(r + 1) * 128], Wv[:, r * 128:(r + 1) * 128]],
            axis=1).astype(bf)                              # (D, 768)
        wch = np.ascontiguousarray(
            wcols.reshape(KT, 128, 768).transpose(1, 0, 2))  # (128,KT,768)
        in_maps.append({
            "xT": xTb[g],
            "wc": wch,
            "wo": wo_t,
            "cosq": cosq, "sinq": sinq, "cosk": cosk, "sink": sink,
            "maskt": maskt,
        })
    return in_maps


def _assemble(results, tb_count=T // 128):
    nhalf = tb_count // 8
    t_tokens = tb_count * 128
    out = np.empty((B, t_tokens, D_MODEL), dtype=np.float32)
    for c in range(N_CORES):
        oc = np.asarray(results[c]["out"],
                        dtype=np.float32)       # (NHALF, B, 128, D)
        for h in range(nhalf):
            t0 = (8 * h + c) * 128
            for b in range(B):
                out[b, t0:t0 + 128, :] = oc[h, b]
    return out


_NC_CACHE = {}


def kernel(x, Wq, Wkv, Wo, q_norm_w, k_norm_w):
    x = np.asarray(x, dtype=np.float32)
    Wq = np.asarray(Wq, dtype=np.float32)
    Wkv = np.asarray(Wkv, dtype=np.float32)
    Wo = np.asarray(Wo, dtype=np.float32)
    q_norm_w = np.asarray(q_norm_w, dtype=np.float32)
    k_norm_w = np.asarray(k_norm_w, dtype=np.float32)

    if "nc" not in _NC_CACHE:
        _NC_CACHE["nc"] = build_nc()
    nc = _NC_CACHE["nc"]
    in_maps = _host_prep(x, Wq, Wkv, Wo, q_norm_w, k_norm_w)
    res = run_bass_kernel_spmd(nc, in_maps, core_ids=list(range(N_CORES)))
    return _assemble(res.results)


if __name__ == "__main__":
    rng = np.random.default_rng(0)
    x = rng.standard_normal((B, T, D_MODEL), dtype=np.float32)
    Wq = rng.standard_normal((D_MODEL, N_Q * D_HEAD), dtype=np.float32) * 0.02
    Wkv = rng.standard_normal((D_MODEL, 2 * N_KV * D_HEAD), dtype=np.float32) * 0.02
    Wo = rng.standard_normal((N_Q * D_HEAD, D_MODEL), dtype=np.float32) * 0.02
    w1 = np.ones(D_HEAD, dtype=np.float32)
    o = kernel(x, Wq, Wkv, Wo, w1, w1)
    print(o.shape, o.dtype, float(np.abs(o).mean()))


# revision 49
# speedup vs baseline: 1.0632x; 1.0632x over previous
"""Trainium2 Bass kernel for GQA attention (B=2,T=2048,D=2048,Hq=16,Hkv=4,Dh=128)
with RMSNorm + YaRN RoPE on q/k, causal softmax, out projection.

Sharding: DP(batch) x TP(heads) hybrid across 8 cores.
  core c -> batch g = c//4, kv group r = c%4, query heads {r, r+4, r+8, r+12}.
  Each core projects q (4 heads) + k + v (its group) for its batch only,
  runs RMSNorm+RoPE, causal attention for its 4 heads, chunk-pipelined with
  the projection.  Two 8-core AllToAlls (one per 8-block half of T)
  redistribute attention outputs from head-sharded to token-sharded: core c
  ends up owning token blocks {c, c+8} of BOTH batches and runs the
  out-projection against full Wo.  Host concatenates token slices.
  Compute in bf16 (f32 PSUM).

v2 scheduling (from trace analysis of v1):
  - gs loads ride gpsimd only: a DMA waiting on the collective must never
    sit on the Scalar queue where it blocks the attention exps.
  - Wo streams in as 8 x 1MB pieces on the (otherwise idle after startup)
    Scalar DMA queue, spread across chunks 0-1, so it neither bursts the
    HBM nor delays the half-0 a2a-input DMAs.
  - proj/rope software pipeline: DVE rope runs right after the projection
    matmuls of its block, but the PE transposes of the roped outputs are
    deferred one chunk so they never wait on the DVE chain.
  - rope processes all 4 query heads in single wide DVE ops (broadcast
    APs for the cos/sin tables and the per-head rsqrt); k-rope, the RMS
    square-accumulates, and all small copies run on the idle Pool
    (gpsimd) engine.
  - score rows are computed in pairs into one 2-bank PSUM tile and
    exponentiated with a single [128,1024] ACT instruction (the causal
    mask is added in-place in PSUM first), halving ACT instruction count.
"""

import math

import numpy as np
import ml_dtypes

import concourse.bass as bass
import concourse.tile as tile
from concourse import bacc, mybir
from concourse.bass_utils import run_bass_kernel_spmd
from concourse.masks import make_identity

# ---- problem constants --------------------------------------------------
B = 2
T = 2048
D_MODEL = 2048
D_HEAD = 128
N_Q, N_KV = 16, 4
ROPE_BASE = 10000.0
YARN_SCALE = 4.0
ORIG_MAX_LEN = 4096
BETA_FAST, BETA_SLOW = 32.0, 1.0
RMS_EPS = 1.1920929e-07
MSCALE = 0.1 * math.log(YARN_SCALE) + 1.0
ATTN_SCALE = 1.0 / (MSCALE * math.sqrt(D_HEAD))

N_CORES = 8
GROUP = 4                    # cores per batch (TP group size)
NH = N_Q // GROUP            # query heads per core
KT = D_MODEL // 128          # 16 contraction tiles
QC = 4                       # query blocks per attention chunk
BF16 = mybir.dt.bfloat16
F32 = mybir.dt.float32
NEG_BIG = -1e30
Alu = mybir.AluOpType
Act = mybir.ActivationFunctionType

# oproj: gs slot idx = s*NH + hh holds global head 4*hh + s
HEAD_OF = [4 * (idx % NH) + idx // NH for idx in range(GROUP * NH)]


# ---- bass graph ---------------------------------------------------------

def build_nc(tb_count=T // 128):
    """One SPMD graph shared by all 8 cores."""
    TB = tb_count
    assert TB % 8 == 0 and TB % QC == 0
    NJ = TB // QC                  # attention chunks
    NHALF = TB // 8                # a2a halves
    t_tokens = TB * 128
    nc = bacc.Bacc("TRN2", target_bir_lowering=False, debug=False,
                   num_devices=N_CORES)

    # all inputs are pre-tiled on the host into exactly the SBUF layouts so
    # every DMA reads large contiguous spans (256B-element gathers run at
    # ~1/3 of HBM bandwidth and starve the PE's SBUF ports)
    xT = nc.dram_tensor("xT", [TB, 128, KT, 128], BF16, kind="ExternalInput")
    wc = nc.dram_tensor("wc", [128, KT, 768], BF16, kind="ExternalInput")
    wo = nc.dram_tensor("wo", [4, 2, 128, 8, 512], BF16,
                        kind="ExternalInput")
    cosq = nc.dram_tensor("cosq", [128, TB, 128], BF16, kind="ExternalInput")
    sinq = nc.dram_tensor("sinq", [128, TB, 128], BF16, kind="ExternalInput")
    cosk = nc.dram_tensor("cosk", [128, TB, 128], BF16, kind="ExternalInput")
    sink = nc.dram_tensor("sink", [128, TB, 128], BF16, kind="ExternalInput")
    maskt = nc.dram_tensor("maskt", [128, 128], F32, kind="ExternalInput")
    out = nc.dram_tensor("out", [NHALF, B, 128, D_MODEL], BF16,
                         kind="ExternalOutput")

    with tile.TileContext(nc) as tc:
        with (
            tc.tile_pool(name="const", bufs=1) as constp,
            tc.tile_pool(name="xin", bufs=2) as xin,
            tc.tile_pool(name="qnp", bufs=8) as qnp,
            tc.tile_pool(name="knp", bufs=8) as knp,
            tc.tile_pool(name="persist", bufs=1) as persist,
            tc.tile_pool(name="work", bufs=2) as work,
            tc.tile_pool(name="outp", bufs=1) as outp,
            tc.tile_pool(name="psum", bufs=1, space="PSUM") as psum,
            tc.tile_pool(name="dram", bufs=1, space="DRAM") as dram,
        ):
            # ---- constants ------------------------------------------------
            # wc first, 4 back-to-back DMAs on the scalar queue (its HW DMA
            # queue is otherwise unused); xt for chunk 0 on sync in parallel
            # tables: first chunk's rows early (small), tails deferred
            cs = {}
            tbl_tails = []
            for name, t in (("cosq", cosq), ("sinq", sinq),
                            ("cosk", cosk), ("sink", sink)):
                s = constp.tile([128, TB, 128], BF16, tag=name, name=name)
                tv = t.ap()
                nc.gpsimd.dma_start(out=s[:, 0:QC, :], in_=tv[:, 0:QC, :])
                tbl_tails.append((s, tv))
                cs[name] = s
            wcs = constp.tile([128, KT, 768], BF16, tag="wc", name="wc")
            wc_dmas = []
            for g in range(KT):
                eng = nc.scalar if g % 2 == 0 else nc.gpsimd
                wd = eng.dma_start(
                    out=wcs[:, g:g + 1, :],
                    in_=wc.ap()[:, g:g + 1, :])
                wc_dmas.append(wd)
            for i, (s, tv) in enumerate(tbl_tails):
                td = nc.gpsimd.dma_start(out=s[:, QC:TB, :],
                                         in_=tv[:, QC:TB, :])
                if i == 0:
                    tile.add_dep_helper(td.ins, wc_dmas[-2].ins, sync=True,
                                        reason="defer table tails")
            mask_sb = constp.tile([128, 128], F32, tag="mask", name="mask")
            nc.gpsimd.dma_start(out=mask_sb, in_=maskt.ap())
            ident = constp.tile([128, 128], BF16, tag="ident", name="ident")
            make_identity(nc, ident)
            eps_sb = constp.tile([128, 1], F32, tag="eps", name="eps")
            nc.vector.memset(eps_sb, RMS_EPS)

            # persistent attention state
            qTall = persist.tile([128, NH, TB * 128], BF16, tag="qT",
                                 name="qT")
            kTt = persist.tile([128, TB, 128], BF16, tag="kT", name="kT")
            vA = persist.tile([128, TB, 129], BF16, tag="vA", name="vA")
            nc.vector.memset(vA[:, :, 128:129], 1.0)

            # full Wo resident in SBUF (4 col-chunks); streamed in as 8
            # 1MB pieces on the scalar queue at points spread over chunks 0-1
            wo_t = [constp.tile([128, N_Q, 512], BF16, tag=f"wo{cc}",
                                name=f"wo{cc}") for cc in range(4)]
            wo_pieces = [(cc, hlf) for cc in range(4) for hlf in range(2)]

            def fire_wo_piece():
                if not wo_pieces:
                    return
                cc, hlf = wo_pieces.pop(0)
                nc.scalar.dma_start(
                    out=wo_t[cc][:, 8 * hlf:8 * (hlf + 1), :],
                    in_=wo.ap()[cc, hlf])

            # ---- A2A bounce buffers (per half) ---------------------------
            a2a_in, a2a_out = {}, {}
            for h in range(NHALF):
                a2a_in[h] = dram.tile([N_CORES, NH, 128, 128], BF16,
                                      tag=f"a2ain{h}", name=f"a2ain{h}")
                a2a_out[h] = dram.tile([N_CORES, NH, 128, 128], BF16,
                                       tag=f"a2aout{h}", name=f"a2aout{h}")

            # ---- projection matmuls + rope (DVE part) for one block ------
            xt_tiles = {}

            def prefetch_xt_chunk(j):
                # half-chunk (2-block) granularity: big enough DMAs, small
                # enough tiles; chunk 0 loads per-block so the first matmul
                # starts asap
                for half in range(2):
                    key = 2 * j + half
                    t0 = j * QC + 2 * half
                    if key in xt_tiles or t0 >= TB:
                        continue
                    xt = xin.tile([128, 2, KT, 128], BF16, tag="xt",
                                  name="xt", bufs=3)
                    if j == 0:
                        for q in range(2):
                            nc.sync.dma_start(out=xt[:, q],
                                              in_=xT.ap()[t0 + q])
                    else:
                        nc.sync.dma_start(
                            out=xt,
                            in_=xT.ap()[t0:t0 + 2]
                                .rearrange("tb p f c -> p tb f c"))
                    xt_tiles[key] = xt

            qn_tiles, kn_tiles = {}, {}

            def proj_mm(tb):
                xt = xt_tiles[tb // 2]
                xtb = tb % 2
                psq = psum.tile([128, NH, 128], F32, tag="pq", name="pq",
                                bufs=2)
                pskv = psum.tile([128, 2, 128], F32, tag="rb", name="pkv",
                                 bufs=2)
                for kt in range(KT):
                    nc.tensor.matmul(psq[:, :, :], xt[:, xtb, kt, :],
                                     wcs[:, kt, 0:512],
                                     start=(kt == 0), stop=(kt == KT - 1))
                for kt in range(KT):
                    nc.tensor.matmul(pskv[:, :, :], xt[:, xtb, kt, :],
                                     wcs[:, kt, 512:768],
                                     start=(kt == 0), stop=(kt == KT - 1))
                # RMS rsqrt: squares+row-sums on ACT (gpsimd can't read
                # PSUM), sqrt ACT, recip DVE; k/v leave PSUM via ACT copies
                sqscr = work.tile([128, 128], BF16, tag="sqscr",
                                   name="sqscr", bufs=1)
                ssq = work.tile([128, 5], F32, tag="ssq", name="ssq")
                for i in range(NH):
                    nc.scalar.activation(sqscr, psq[:, i, :], Act.Square,
                                         accum_out=ssq[:, i:i + 1])
                nc.scalar.activation(sqscr, pskv[:, 0, :], Act.Square,
                                     accum_out=ssq[:, 4:5])
                nc.scalar.activation(vA[:, tb, 0:128], pskv[:, 1, :],
                                     Act.Copy)
                std = work.tile([128, 5], F32, tag="std", name="std")
                nc.scalar.activation(std, ssq, Act.Sqrt,
                                     bias=eps_sb, scale=1.0 / 128.0)
                rv = work.tile([128, 5], F32, tag="rv", name="rv")
                nc.vector.reciprocal(rv, std)
                # normalized k to SBUF in one ACT copy (scale = rsqrt)
                ksb = work.tile([128, 128], BF16, tag="ksb", name="ksb")
                nc.scalar.activation(ksb, pskv[:, 0, :], Act.Copy,
                                     scale=rv[:, 4:5])
                # q rope: all 4 heads in wide DVE ops
                rvb = rv[:, 0:4, None].broadcast_to((128, 4, 128))
                cosb = cs["cosq"][:, None, tb, :].broadcast_to((128, 4, 128))
                sint = cs["sinq"][:, tb, :]
                qrv = work.tile([128, 4, 128], BF16, tag="qrv",
                                 name="qrv", bufs=1)
                nc.vector.tensor_tensor(qrv, psq[:, :, :], rvb, Alu.mult)
                t1 = work.tile([128, 4, 128], BF16, tag="t1", name="t1",
                                bufs=1)
                nc.vector.tensor_tensor(t1, qrv, cosb, Alu.mult)
                t2 = work.tile([128, 4, 128], BF16, tag="t2", name="t2",
                                bufs=1)
                nc.vector.tensor_tensor(
                    t2[:, :, 0:64], qrv[:, :, 64:128],
                    sint[:, None, 0:64].broadcast_to((128, 4, 64)), Alu.mult)
                nc.vector.tensor_tensor(
                    t2[:, :, 64:128], qrv[:, :, 0:64],
                    sint[:, None, 64:128].broadcast_to((128, 4, 64)),
                    Alu.mult)
                qn = qnp.tile([128, 4, 128], BF16, tag="qn", name="qn")
                nc.vector.tensor_tensor(qn, t1, t2, Alu.add)
                qn_tiles[tb] = qn
                # k rope on gpsimd from the normalized SBUF copy
                t1k = work.tile([128, 128], BF16, tag="t1k", name="t1k")
                nc.gpsimd.tensor_tensor(t1k, ksb, cs["cosk"][:, tb, :],
                                        Alu.mult)
                t2k = work.tile([128, 128], BF16, tag="t2k", name="t2k")
                nc.gpsimd.tensor_tensor(t2k[:, 0:64], ksb[:, 64:128],
                                        cs["sink"][:, tb, 0:64], Alu.mult)
                nc.gpsimd.tensor_tensor(t2k[:, 64:128], ksb[:, 0:64],
                                        cs["sink"][:, tb, 64:128], Alu.mult)
                kn = knp.tile([128, 128], BF16, tag="kn", name="kn")
                nc.gpsimd.tensor_tensor(kn, t1k, t2k, Alu.add)
                kn_tiles[tb] = kn

            # ---- deferred PE transposes for one block --------------------
            def rope_transpose(tb):
                qn = qn_tiles.pop(tb)
                trp = psum.tile([128, 4, 128], BF16, tag="rb", name="trp",
                                bufs=2)
                for hh in range(NH):
                    nc.tensor.transpose(trp[:, hh, :], qn[:, hh, :], ident)
                nc.vector.tensor_copy(
                    qTall[:, :, tb * 128:(tb + 1) * 128], trp[:, :, :])
                kn = kn_tiles.pop(tb)
                trpk = psum.tile([128, 128], BF16, tag="rb", name="trpk",
                                 bufs=2)
                nc.tensor.transpose(trpk, kn, ident)
                nc.vector.tensor_copy(kTt[:, tb, :], trpk)

            # ---- attention chunk j for head hh ---------------------------
            # Score rows are processed in PAIRS: both rows' matmuls land in
            # one [128,2,512] PSUM tile (2 banks), the causal mask is added
            # in place, then a single [128,1024] exp produces the pair's
            # probabilities.  Two-pass pa accumulators as in v1: pass A
            # runs pv for q-blocks 0,1 while scoring, pass B replays pv for
            # q-blocks 2,3 from the saved exp tiles.
            def attn_block(j, hh):
                q0 = j * QC
                nrows = q0 + QC
                qcols = qTall[:, hh, q0 * 128:(q0 + QC) * 128]
                pas = {}

                def alloc_pa(qq):
                    pas[qq] = psum.tile([128, 129], F32, tag="rb", name="pa",
                                        bufs=2)

                def score_pair(p):
                    ss = psum.tile([128, 2, 512], F32, tag="ps", name="ps",
                                   bufs=2)
                    for par in range(2):
                        kb = 2 * p + par
                        diag = kb >= q0
                        w = (q0 + QC - kb) if diag else QC
                        cols = qTall[:, hh, (kb if diag else q0) * 128:
                                     (q0 + QC) * 128]
                        nc.tensor.matmul(ss[:, par, 0:w * 128], kTt[:, kb, :],
                                         cols, start=True, stop=True)
                        if diag:
                            nc.vector.scalar_tensor_tensor(
                                out=ss[:, par, 0:128], in0=ss[:, par, 0:128],
                                scalar=1.0, in1=mask_sb,
                                op0=Alu.bypass, op1=Alu.add)
                    ptw = work.tile([128, 2, 512], BF16, tag="ptw",
                                    name="ptw", bufs=8)
                    nc.scalar.activation(ptw[:, :, :], ss[:, :, :], Act.Exp,
                                         scale=ATTN_SCALE)
                    return ptw

                def pv(p, ptw, qlo, qhi):
                    for par in range(2):
                        kb = 2 * p + par
                        if kb >= nrows:
                            continue
                        lo = kb - q0 if kb >= q0 else 0
                        for qq in range(max(qlo, lo), qhi):
                            off = qq - lo
                            nc.tensor.matmul(
                                pas[qq],
                                ptw[:, par, off * 128:(off + 1) * 128],
                                vA[:, kb, :], start=(kb == 0),
                                stop=(q0 + qq == kb))

                at4 = work.tile([128, QC, 128], BF16, tag="attnT",
                                name="attnT", bufs=2)

                def tail(qq):
                    pa = pas[qq]
                    rs = work.tile([128, 1], F32, tag="rsum", name="rsum")
                    nc.vector.reciprocal(rs, pa[:, 128:129])
                    an = work.tile([128, 128], BF16, tag="attn_n",
                                   name="attn_n")
                    nc.vector.tensor_scalar_mul(an, pa[:, 0:128], rs)
                    tr = psum.tile([128, 128], BF16, tag="rb", name="trt",
                                   bufs=2)
                    nc.tensor.transpose(tr, an, ident)
                    nc.vector.tensor_copy(at4[:, qq, :], tr)

                def flush_tails():
                    # one DMA moves all 4 query blocks of this (chunk, head)
                    nc.sync.dma_start(
                        out=a2a_in[q0 // 8][4 * ((q0 // 4) % 2):
                                            4 * ((q0 // 4) % 2) + 4, hh]
                            .rearrange("s p t -> p s t"),
                        in_=at4)

                npairs = (nrows + 1) // 2
                alloc_pa(0)
                alloc_pa(1)
                ptws = {}
                for p in range(npairs):
                    ptws[p] = score_pair(p)
                    if p > 0:
                        pv(p - 1, ptws[p - 1], 0, 2)
                pv(npairs - 1, ptws[npairs - 1], 0, 2)
                tail(0)
                tail(1)
                alloc_pa(2)
                alloc_pa(3)
                for p in range(npairs):
                    pv(p, ptws[p], 2, 4)
                tail(2)
                tail(3)
                flush_tails()

            def fire_a2a(h):
                nc.gpsimd.collective_compute(
                    "AllToAll", Alu.bypass,
                    replica_groups=[list(range(N_CORES))],
                    ins=[a2a_in[h].opt()], outs=[a2a_out[h].opt()])

            # ---- out-projection for one half -----------------------------
            gs_tiles = {}

            def load_gs(h):
                for b in range(B):
                    g = persist.tile([128, GROUP, NH, 128], BF16,
                                     tag="gs", name="gs", bufs=3)
                    gs_tiles[(h, b)] = g
                    nc.gpsimd.dma_start(
                        out=g,
                        in_=a2a_out[h][GROUP * b:GROUP * (b + 1)]
                            .rearrange("s hh p t -> p (s hh) t"))

            def oproj(h):
                gs = {b: gs_tiles[(h, b)] for b in range(B)}
                ob = {b: outp.tile([128, 4, 512], BF16, tag=f"os{b}",
                                   name=f"os{b}") for b in range(B)}
                for cc in range(4):
                    for b in range(B):
                        po = psum.tile([128, 512], F32, tag="ps",
                                       name="po", bufs=2)
                        for idx in range(GROUP * NH):
                            nc.tensor.matmul(po,
                                             gs[b][:, idx // NH, idx % NH, :],
                                             wo_t[cc][:, HEAD_OF[idx], :],
                                             start=(idx == 0),
                                             stop=(idx == GROUP * NH - 1))
                        nc.scalar.activation(ob[b][:, cc, :], po, Act.Copy)
                for b in range(B):
                    nc.sync.dma_start(out=out.ap()[h, b], in_=ob[b])

            # ---- emission: iter it does [transposes(it-1)] [proj(it)]
            # [attn(it-1)]; a2a for a half fires when its last chunk ends.
            prefetch_xt_chunk(0)
            for it in range(NJ + 1):
                if it > 0:
                    for tb in range((it - 1) * QC, it * QC):
                        rope_transpose(tb)
                if it < NJ:
                    for tb in range(it * QC, (it + 1) * QC):
                        proj_mm(tb)
                    prefetch_xt_chunk(it + 1)
                if it > 0:
                    j = it - 1
                    for hh in range(NH):
                        attn_block(j, hh)
                        if j < 2:
                            fire_wo_piece()
                    if ((j + 1) * QC) % 8 == 0:
                        h = ((j + 1) * QC) // 8 - 1
                        fire_a2a(h)
                        load_gs(h)
                        if h == NHALF - 1:
                            for hq in range(NHALF):
                                oproj(hq)
    nc.compile()
    return nc


# ---- host side ----------------------------------------------------------

def _yarn_tables(t_tokens):
    inv = 1.0 / ROPE_BASE ** (np.arange(0, D_HEAD, 2, dtype=np.float32) / D_HEAD)
    wavelengths = 2.0 * math.pi / inv
    low_wl = ORIG_MAX_LEN / BETA_SLOW
    high_wl = ORIG_MAX_LEN / BETA_FAST
    gamma = np.clip((low_wl - wavelengths) / (low_wl - high_wl), 0.0, 1.0)
    inv_freq = (gamma * inv + (1.0 - gamma) * inv / YARN_SCALE).astype(np.float32)
    t = np.arange(t_tokens, dtype=np.float32)
    freqs = np.outer(t, inv_freq)                      # (T, 64)
    emb = np.concatenate([freqs, freqs], axis=-1)      # (T, 128)
    return np.cos(emb).astype(np.float32), np.sin(emb).astype(np.float32)


def _host_prep(x, Wq, Wkv, Wo, q_norm_w, k_norm_w, tb_count=T // 128):
    TB = tb_count
    t_tokens = TB * 128
    bf = ml_dtypes.bfloat16

    def tile_tokens(a):
        # (t_tokens, 128) f32 -> (128 p, TB, 128) pre-tiled
        return np.ascontiguousarray(
            a.reshape(TB, 128, 128).transpose(1, 0, 2)).astype(bf)

    xTb = []
    for b in range(B):
        xT = x[b, :t_tokens, :].T                          # (D, T)
        xt = xT.reshape(KT, 128, TB, 128).transpose(2, 1, 0, 3)
        xTb.append(np.ascontiguousarray(xt).astype(bf))    # (TB,128,KT,128)
    cos, sin = _yarn_tables(t_tokens)
    sinF = sin.copy()
    sinF[:, :64] *= -1.0
    # rms weight applies to x before rope; the sin term reads the *rotated*
    # input, so its weight index is the input position (rolled by 64).
    wq_roll = np.concatenate([q_norm_w[64:], q_norm_w[:64]])
    wk_roll = np.concatenate([k_norm_w[64:], k_norm_w[:64]])
    cosq = tile_tokens(cos * q_norm_w[None, :])
    sinq = tile_tokens(sinF * wq_roll[None, :])
    cosk = tile_tokens(cos * k_norm_w[None, :])
    sink = tile_tokens(sinF * wk_roll[None, :])
    maskt = np.where(np.arange(128)[:, None] <= np.arange(128)[None, :],
                     0.0, NEG_BIG).astype(np.float32)       # [k, q]
    Wk, Wv = Wkv[:, :N_KV * D_HEAD], Wkv[:, N_KV * D_HEAD:]
    # wo pieces: [cc, half, p, head, n] contiguous 1MB pieces
    wo_t = np.ascontiguousarray(
        Wo.astype(bf).reshape(2, 8, 128, 4, 512).transpose(3, 0, 2, 1, 4))
    in_maps = []
    for c in range(N_CORES):
        g, r = c // GROUP, c % GROUP
        wcols = np.concatenate(
            [Wq[:, h * 128:(h + 1) * 128]
             for h in (r, r + 4, r + 8, r + 12)]
            + [Wk[:, r * 128:(r + 1) * 128], Wv[:, r * 128:(r + 1) * 128]],
            axis=1).astype(bf)                              # (D, 768)
        wch = np.ascontiguousarray(
            wcols.reshape(KT, 128, 768).transpose(1, 0, 2))  # (128,KT,768)
        in_maps.append({
            "xT": xTb[g],
            "wc": wch,
            "wo": wo_t,
            "cosq": cosq, "sinq": sinq, "cosk": cosk, "sink": sink,
            "maskt": maskt,
        })
    return in_maps


def _assemble(results, tb_count=T // 128):
    nhalf = tb_count // 8
    t_tokens = tb_count * 128
    out = np.empty((B, t_tokens, D_MODEL), dtype=np.float32)
    for c in range(N_CORES):
        oc = np.asarray(results[c]["out"],
                        dtype=np.float32)       # (NHALF, B, 128, D)
        for h in range(nhalf):
            t0 = (8 * h + c) * 128
            for b in range(B):
                out[b, t0:t0 + 128, :] = oc[h, b]
    return out


_NC_CACHE = {}


def kernel(x, Wq, Wkv, Wo, q_norm_w, k_norm_w):
    x = np.asarray(x, dtype=np.float32)
    Wq = np.asarray(Wq, dtype=np.float32)
    Wkv = np.asarray(Wkv, dtype=np.float32)
    Wo = np.asarray(Wo, dtype=np.float32)
    q_norm_w = np.asarray(q_norm_w, dtype=np.float32)
    k_norm_w = np.asarray(k_norm_w, dtype=np.float32)

    if "nc" not in _NC_CACHE:
        _NC_CACHE["nc"] = build_nc()
    nc = _NC_CACHE["nc"]
    in_maps = _host_prep(x, Wq, Wkv, Wo, q_norm_w, k_norm_w)
    res = run_bass_kernel_spmd(nc, in_maps, core_ids=list(range(N_CORES)))
    return _assemble(res.results)


if __name__ == "__main__":
    rng = np.random.default_rng(0)
    x = rng.standard_normal((B, T, D_MODEL), dtype=np.float32)
    Wq = rng.standard_normal((D_MODEL, N_Q * D_HEAD), dtype=np.float32) * 0.02
    Wkv = rng.standard_normal((D_MODEL, 2 * N_KV * D_HEAD), dtype=np.float32) * 0.02
    Wo = rng.standard_normal((N_Q * D_HEAD, D_MODEL), dtype=np.float32) * 0.02
    w1 = np.ones(D_HEAD, dtype=np.float32)
    o = kernel(x, Wq, Wkv, Wo, w1, w1)
    print(o.shape, o.dtype, float(np.abs(o).mean()))
